# revision 2
# baseline (speedup 1.0000x reference)
"""DeepClusteringLoss on 8 TRN2 NeuronCores.

loss = -sum_b ||E_b^T Y_b||_F^2 / (mean_b ||E_b^T E_b||_F^2 + 1e-8)
with Y = V / (colsum(V) + 1e-8), E: (B, N, D), V: (B, N, S), N = F*T.

Sharding: data-parallel over batch (8 batches -> 8 cores). Each core
reduces its 22.6 MB shard to a (110,110) Gram block matrix + (1,240)
column-sum vector on-device; the host sums diagonal blocks and combines
the per-batch scalars.

Device algorithm (per core):
  Host pre-interleaves E and V into EV = (N, 22) rows [e_0..e_19, v_0, v_1]
  and zero-pads N to 128*120*17 = 261120 (zero rows change nothing).
  The padded array is viewed as (17, 128, 120*22): group g, partition p,
  then 120 row-chunks of 22 values within the partition line (so each
  DMA line is 120*22*4 = 10560 contiguous bytes in DRAM).
  Per group one SWDGE DMA loads (128, 2640) f32 -> bf16 (cast in-flight;
  HBM still reads the full fp32 bytes).
  Matmuls contract over the 128 partitions: taking a 110-column slice
  (5 chunks x 22) as both stationary and moving gives a (110,110) PSUM
  block whose 5 diagonal 22x22 blocks are sum_{rows} [e|v]^T [e|v] =
  [[E^T E, E^T V], [V^T E, V^T V]] for those rows.  Accumulating over
  all 24 slices x 17 groups leaves the full-batch Gram sums in the
  diagonal blocks (off-diagonal blocks are unused cross terms).
  colsum(V) comes from ones(128,1)^T @ V-columns (strided AP), PSUM-
  accumulated to (1, 240) = per-(chunk, s) partial sums.
"""

import sys

if "/opt/trn_rl_repo" not in sys.path:
    sys.path.insert(0, "/opt/trn_rl_repo")

from contextlib import ExitStack

import numpy as np

import concourse.bass as bass
import concourse.tile as tile
from concourse import bacc, mybir
from concourse.bass_utils import run_bass_kernel_spmd

# Problem geometry (hardcoded; see spec)
B, F, T, D, S = 8, 257, 1000, 20, 2
N = F * T  # 257000
CH = D + S  # 22 interleaved columns per row
P = 128  # SBUF partitions

# Tiling: NPAD = P * M * G rows per core, C chunks per matmul
M = 120  # row-chunks per partition per group
G = 17  # groups (one DMA each)
C = 5  # chunks fused per matmul (C*CH = 110 <= 128 stationary cols)
NPAD = P * M * G  # 261120
INNER = M // C  # 24 matmuls per group
BLK = C * CH  # 110


def build_bass(m=M, g=G, n_cores=B):
    """Build + compile the per-core Bass program (same SPMD program on
    every core; only the input data differs)."""
    inner = m // C
    npad = P * m * g
    nc = bacc.Bacc(
        "TRN2", target_bir_lowering=False, debug=False, num_devices=n_cores
    )
    ev = nc.dram_tensor("ev", [npad, CH], mybir.dt.float32, kind="ExternalInput")
    out_g = nc.dram_tensor("out_g", [BLK, BLK], mybir.dt.float32, kind="ExternalOutput")
    out_cs = nc.dram_tensor(
        "out_cs", [1, m * S], mybir.dt.float32, kind="ExternalOutput"
    )

    with tile.TileContext(nc) as tc, ExitStack() as ctx:
        evpool = ctx.enter_context(tc.tile_pool(name="ev", bufs=3))
        const = ctx.enter_context(tc.tile_pool(name="const", bufs=1))
        psum = ctx.enter_context(tc.tile_pool(name="acc", bufs=1, space="PSUM"))
        sbout = ctx.enter_context(tc.tile_pool(name="sbout", bufs=1))

        ones = const.tile([P, 1], mybir.dt.bfloat16)
        nc.gpsimd.memset(ones[:], 1.0)

        gacc = psum.tile([BLK, BLK], mybir.dt.float32)
        csacc = psum.tile([1, m * S], mybir.dt.float32)

        ev_all = ev.ap().rearrange("(g p m) d -> g p (m d)", g=g, p=P)
        for gi in range(g):
            evt = evpool.tile([P, m * CH], mybir.dt.bfloat16)
            # SWDGE DMA with fp32 -> bf16 cast in flight
            nc.gpsimd.dma_start(out=evt[:], in_=ev_all[gi])
            for j in range(inner):
                sl = evt[:, j * BLK : (j + 1) * BLK]
                nc.tensor.matmul(
                    gacc[:],
                    sl,
                    sl,
                    start=(gi == 0 and j == 0),
                    stop=(gi == g - 1 and j == inner - 1),
                )
            # V columns of every chunk, strided: (128, m, S) slice
            vs = evt[:].rearrange("p (m d) -> p m d", d=CH)[:, :, D:CH]
            nc.tensor.matmul(
                csacc[:], ones[:], vs, start=(gi == 0), stop=(gi == g - 1)
            )

        gsb = sbout.tile([BLK, BLK], mybir.dt.float32)
        nc.vector.tensor_copy(gsb[:], gacc[:])
        nc.sync.dma_start(out=out_g.ap(), in_=gsb[:])
        cssb = sbout.tile([1, m * S], mybir.dt.float32)
        nc.vector.tensor_copy(cssb[:], csacc[:])
        nc.sync.dma_start(out=out_cs.ap(), in_=cssb[:])

    nc.compile()
    return nc


def pack_inputs(embeddings, source_indicators, m=M, g=G):
    """(B,F,T,D)+(B,F,T,S) -> per-core padded interleaved (NPAD, 22)."""
    b = embeddings.shape[0]
    n = embeddings.shape[1] * embeddings.shape[2]
    npad = P * m * g
    e = np.asarray(embeddings, dtype=np.float32).reshape(b, n, D)
    v = np.asarray(source_indicators, dtype=np.float32).reshape(b, n, S)
    evp = np.zeros((b, npad, CH), dtype=np.float32)
    evp[:, :n, :D] = e
    evp[:, :n, D:] = v
    return evp


def reduce_outputs(res, m=M):
    """Per-core raw outputs -> (G_b, EtV_b, colsum_b) in float64."""
    out_g = np.asarray(res["out_g"], dtype=np.float64)
    out_cs = np.asarray(res["out_cs"], dtype=np.float64)
    g_b = np.zeros((D, D))
    etv_b = np.zeros((D, S))
    for c in range(C):
        blk = out_g[c * CH : (c + 1) * CH, c * CH : (c + 1) * CH]
        g_b += blk[:D, :D]
        etv_b += blk[:D, D:CH]
    colsum_b = out_cs.reshape(m, S).sum(axis=0)
    return g_b, etv_b, colsum_b


_NC_CACHE = {}


def _get_nc():
    if "nc" not in _NC_CACHE:
        _NC_CACHE["nc"] = build_bass()
    return _NC_CACHE["nc"]


def kernel(embeddings, source_indicators):
    evp = pack_inputs(embeddings, source_indicators)
    nc = _get_nc()
    in_maps = [{"ev": np.ascontiguousarray(evp[b])} for b in range(B)]
    results = run_bass_kernel_spmd(nc, in_maps, list(range(B))).results

    loss = 0.0
    norms = []
    for b in range(B):
        g_b, etv_b, colsum_b = reduce_outputs(results[b])
        ety = etv_b / (colsum_b[None, :] + 1e-8)
        loss += float(np.sum(ety * ety))
        norms.append(float(np.sum(g_b * g_b)))
    norm_term = float(np.mean(norms))
    return np.float32(-loss / (norm_term + 1e-8))


# revision 7
# speedup vs baseline: 1.0449x; 1.0449x over previous
"""DeepClusteringLoss on 8 TRN2 NeuronCores.

loss = -sum_b ||E_b^T Y_b||_F^2 / (mean_b ||E_b^T E_b||_F^2 + 1e-8)
with Y = V / (colsum(V) + 1e-8), E: (B, N, D), V: (B, N, S), N = F*T.

Sharding: data-parallel over batch (8 batches -> 8 cores). Each core
reduces its 22.6 MB shard to a (110,110) Gram block matrix + (1,510)
column-sum vector on-device; the host sums diagonal blocks and combines
the per-batch scalars (a few hundred flops).

Device algorithm (per core), raw Bass (no Tile framework -> no multi-us
preamble/drain barriers):
  Host pre-interleaves E and V into EV = (N, 22) rows [e_0..e_19, v_0, v_1]
  and zero-pads N=257000 to 2010*128 = 257280 rows (zero rows are inert).
  The padded array is split into DMA groups of m_i row-chunks
  (ms = [510, 510, 510, 425, 55]; sum = 2010; the small last group keeps
  the tensor-engine tail exposed after the final DMA short).  Group i is
  viewed as (128, m_i*22): partition p holds m_i consecutive 22-float rows,
  so each DMA line is m_i*88 contiguous DRAM bytes.  One SWDGE DMA per
  group loads f32 -> bf16 (cast in flight; HBM still reads full fp32).
  Matmuls contract over the 128 partitions: a 110-column slice (5 chunks
  x 22) as both stationary and moving gives a (110,110) PSUM block whose
  five diagonal 22x22 blocks are sum_rows [e|v]^T [e|v] = [[E^T E, E^T V],
  [., V^T V]] for those rows; PSUM-accumulating all slices of all groups
  leaves full-batch Gram sums in the diagonal blocks.  colsum(V) comes
  from ones(128,1)^T @ V-columns (strided AP over <=255 chunks at a
  time), PSUM-accumulated into (1, 510).
"""

import sys

if "/opt/trn_rl_repo" not in sys.path:
    sys.path.insert(0, "/opt/trn_rl_repo")

from contextlib import ExitStack

import numpy as np

import concourse.bass as bass
from concourse import mybir
from concourse.bass_utils import run_bass_kernel_spmd

# Problem geometry (hardcoded; see spec)
B, F, T, D, S = 8, 257, 1000, 20, 2
N = F * T  # 257000
CH = D + S  # 22 interleaved columns per row
P = 128  # SBUF partitions
C = 5  # row-chunks fused per matmul (C*CH = 110 <= 128 stationary cols)
BLK = C * CH  # 110

MS = [510, 510, 510, 425, 55]  # row-chunks per DMA group; sum*P = NPAD
NBUF = 3  # SBUF buffers for DMA/compute overlap
CS_MAX = 255  # chunks per colsum matmul (2*255 <= 512 fp32 PSUM bank)
NPAD = P * sum(MS)  # 257280


def build_bass(ms=None, n_cores=B):
    """Build the per-core raw-Bass SPMD program (same program on every
    core; only the input data differs)."""
    ms = list(MS if ms is None else ms)
    assert all(m % C == 0 for m in ms)
    npad = P * sum(ms)
    mmax = max(ms)
    cs_cols = min(mmax, CS_MAX) * S
    ngrp = len(ms)

    nc = bass.Bass("TRN2", debug=False, num_devices=n_cores)
    ev = nc.dram_tensor("ev", [npad, CH], mybir.dt.float32, kind="ExternalInput")
    out_g = nc.dram_tensor("out_g", [BLK, BLK], mybir.dt.float32, kind="ExternalOutput")
    out_cs = nc.dram_tensor(
        "out_cs", [1, cs_cols], mybir.dt.float32, kind="ExternalOutput"
    )

    # DRAM views per group: (128, m*CH), partition-major rows
    bases = np.cumsum([0] + ms).tolist()
    ev_views = [
        ev.ap()[P * bases[i] : P * bases[i + 1], :].rearrange(
            "(p m) d -> p (m d)", p=P
        )
        for i in range(ngrp)
    ]

    with ExitStack() as ctx:
        bufs = [
            ctx.enter_context(
                nc.sbuf_tensor(f"buf{i}", [P, mmax * CH], mybir.dt.bfloat16)
            )
            for i in range(NBUF)
        ]
        ones = ctx.enter_context(nc.sbuf_tensor("ones", [P, 1], mybir.dt.bfloat16))
        gsb = ctx.enter_context(nc.sbuf_tensor("gsb", [BLK, BLK], mybir.dt.float32))
        cssb = ctx.enter_context(
            nc.sbuf_tensor("cssb", [1, cs_cols], mybir.dt.float32)
        )
        gacc = ctx.enter_context(
            nc.psum_tensor("gacc", [BLK, BLK], mybir.dt.float32)
        )
        csacc = ctx.enter_context(
            nc.psum_tensor("csacc", [1, cs_cols], mybir.dt.float32)
        )
        dma_sems = [
            ctx.enter_context(nc.semaphore(f"dma_sem{i}")) for i in range(NBUF)
        ]
        ten_sem = ctx.enter_context(nc.semaphore("ten_sem"))
        ones_sem = ctx.enter_context(nc.semaphore("ones_sem"))
        copy_sem = ctx.enter_context(nc.semaphore("copy_sem"))
        odma_sem = ctx.enter_context(nc.semaphore("odma_sem"))
        block = ctx.enter_context(nc.Block())

        @block.gpsimd
        def _(g: bass.BassEngine):
            g.memset(ones.ap(), 1.0).then_inc(ones_sem, 1)
            for i, m in enumerate(ms):
                if i >= NBUF:
                    # wait until tensor is done reading this buffer
                    g.wait_ge(ten_sem, i - NBUF + 1)
                buf = bufs[i % NBUF]
                # SWDGE DMA with fp32 -> bf16 cast in flight.  One
                # semaphore per buffer slot: a slot's sem has only one
                # DMA in flight at a time, so sem == 16*(k+1) proves
                # that DMA fully landed (per-SDMA-engine increments of
                # concurrent DMAs interleave on a shared sem).
                g.dma_start(out=buf.ap()[:, : m * CH], in_=ev_views[i]).then_inc(
                    dma_sems[i % NBUF], 16
                )

        @block.tensor
        def _(t: bass.BassEngine):
            t.wait_ge(ones_sem, 1)
            total_g = sum(m // C for m in ms)
            total_cs = sum((m + CS_MAX - 1) // CS_MAX for m in ms)
            gi = ci = 0
            for i, m in enumerate(ms):
                t.wait_ge(dma_sems[i % NBUF], 16 * (i // NBUF + 1))
                buf = bufs[i % NBUF]
                last = None
                for j in range(m // C):
                    sl = buf.ap()[:, j * BLK : (j + 1) * BLK]
                    last = t.matmul(
                        gacc.ap(),
                        sl,
                        sl,
                        start=(gi == 0),
                        stop=(gi == total_g - 1),
                    )
                    gi += 1
                bview = buf.ap()[:, : m * CH].rearrange("p (m d) -> p m d", d=CH)
                for c0 in range(0, m, CS_MAX):
                    cn = min(CS_MAX, m - c0)
                    vs = bview[:, c0 : c0 + cn, D:CH]
                    last = t.matmul(
                        csacc.ap()[:, : cn * S],
                        ones.ap(),
                        vs,
                        start=(ci == 0),
                        stop=(ci == total_cs - 1),
                    )
                    ci += 1
                last.then_inc(ten_sem, 1)

        @block.vector
        def _(v: bass.BassEngine):
            v.wait_ge(ten_sem, ngrp)
            v.tensor_copy(gsb.ap(), gacc.ap())
            v.tensor_copy(cssb.ap(), csacc.ap()).then_inc(copy_sem, 1)

        @block.sync
        def _(s: bass.BassEngine):
            s.wait_ge(copy_sem, 1)
            s.dma_start(out=out_g.ap(), in_=gsb.ap()).then_inc(odma_sem, 16)
            s.dma_start(out=out_cs.ap(), in_=cssb.ap()).then_inc(odma_sem, 16)
            s.wait_ge(odma_sem, 32)

    return nc


def pack_inputs(embeddings, source_indicators, npad=NPAD):
    """(B,F,T,D)+(B,F,T,S) -> per-core padded interleaved (npad, 22)."""
    b = embeddings.shape[0]
    n = embeddings.shape[1] * embeddings.shape[2]
    e = np.asarray(embeddings, dtype=np.float32).reshape(b, n, D)
    v = np.asarray(source_indicators, dtype=np.float32).reshape(b, n, S)
    evp = np.zeros((b, npad, CH), dtype=np.float32)
    evp[:, :n, :D] = e
    evp[:, :n, D:] = v
    return evp


def reduce_outputs(res):
    """Per-core raw outputs -> (G_b, EtV_b, colsum_b) in float64."""
    out_g = np.asarray(res["out_g"], dtype=np.float64)
    out_cs = np.asarray(res["out_cs"], dtype=np.float64)
    g_b = np.zeros((D, D))
    etv_b = np.zeros((D, S))
    for c in range(C):
        blk = out_g[c * CH : (c + 1) * CH, c * CH : (c + 1) * CH]
        g_b += blk[:D, :D]
        etv_b += blk[:D, D:CH]
    colsum_b = out_cs.reshape(-1, S).sum(axis=0)
    return g_b, etv_b, colsum_b


_NC_CACHE = {}


def _get_nc():
    if "nc" not in _NC_CACHE:
        _NC_CACHE["nc"] = build_bass()
    return _NC_CACHE["nc"]


def kernel(embeddings, source_indicators):
    evp = pack_inputs(embeddings, source_indicators)
    nc = _get_nc()
    in_maps = [{"ev": np.ascontiguousarray(evp[b])} for b in range(B)]
    results = run_bass_kernel_spmd(nc, in_maps, list(range(B))).results

    loss = 0.0
    norms = []
    for b in range(B):
        g_b, etv_b, colsum_b = reduce_outputs(results[b])
        ety = etv_b / (colsum_b[None, :] + 1e-8)
        loss += float(np.sum(ety * ety))
        norms.append(float(np.sum(g_b * g_b)))
    norm_term = float(np.mean(norms))
    return np.float32(-loss / (norm_term + 1e-8))


# revision 15
# speedup vs baseline: 1.0839x; 1.0373x over previous
"""DeepClusteringLoss on 8 TRN2 NeuronCores.

loss = -sum_b ||E_b^T Y_b||_F^2 / (mean_b ||E_b^T E_b||_F^2 + 1e-8)
with Y = V / (colsum(V) + 1e-8), E: (B, N, D), V: (B, N, S), N = F*T.

Sharding: data-parallel over batch (8 batches -> 8 cores). Each core
reduces its 22.6 MB shard to a (110,110) Gram block matrix + (1,510)
column-sum vector on-device; the host sums diagonal blocks and combines
the per-batch scalars (a few hundred flops).

Device algorithm (per core), raw Bass (no Tile framework -> no multi-us
preamble/drain barriers):
  Host pre-interleaves E and V into EV = (N, 22) rows [e_0..e_19, v_0, v_1]
  and zero-pads N=257000 to 2010*128 = 257280 rows (zero rows are inert).
  The padded array is split into DMA groups of m_i row-chunks
  (ms = [510, 510, 510, 425, 55]; sum = 2010; the small last group keeps
  the tensor-engine tail exposed after the final DMA short).  Group i is
  viewed as (128, m_i*22): partition p holds m_i consecutive 22-float rows,
  so each DMA line is m_i*88 contiguous DRAM bytes.  One SWDGE DMA per
  group loads f32 -> bf16 (cast in flight; HBM still reads full fp32).
  Matmuls contract over the 128 partitions: a 110-column slice (5 chunks
  x 22) as both stationary and moving gives a (110,110) PSUM block whose
  five diagonal 22x22 blocks are sum_rows [e|v]^T [e|v] = [[E^T E, E^T V],
  [., V^T V]] for those rows; PSUM-accumulating all slices of all groups
  leaves full-batch Gram sums in the diagonal blocks.  colsum(V) comes
  from ones(128,1)^T @ V-columns (strided AP over <=255 chunks at a
  time), PSUM-accumulated into (1, 510).
"""

import sys

if "/opt/trn_rl_repo" not in sys.path:
    sys.path.insert(0, "/opt/trn_rl_repo")

from contextlib import ExitStack

import numpy as np

import concourse.bass as bass
from concourse import mybir
from concourse.bass_utils import run_bass_kernel_spmd

# Problem geometry (hardcoded; see spec)
B, F, T, D, S = 8, 257, 1000, 20, 2
N = F * T  # 257000
CH = D + S  # 22 interleaved columns per row
P = 128  # SBUF partitions
C = 5  # row-chunks fused per matmul (C*CH = 110 <= 128 stationary cols)
BLK = C * CH  # 110

# Row-chunks per DMA group (sum = 2010 -> NPAD = 257280, 0.1% padding).
# Medium groups keep the tensor engine close behind the DMA stream; the
# tiny final groups make the post-DMA matmul tail negligible.
MS = [255] * 7 + [195, 20, 10]
NBUF = 3  # SBUF buffers for DMA/compute overlap
CS_MAX = 255  # chunks per colsum matmul (2*255 <= 512 fp32 PSUM bank)
NPAD = P * sum(MS)  # 257280


def build_bass(ms=None, n_cores=B):
    """Build the per-core raw-Bass SPMD program (same program on every
    core; only the input data differs)."""
    ms = list(MS if ms is None else ms)
    assert all(m % C == 0 for m in ms)
    npad = P * sum(ms)
    mmax = max(ms)
    cs_cols = min(mmax, CS_MAX) * S
    ngrp = len(ms)

    nc = bass.Bass("TRN2", debug=False, num_devices=n_cores)
    ev = nc.dram_tensor("ev", [npad, CH], mybir.dt.float32, kind="ExternalInput")
    out_g = nc.dram_tensor("out_g", [BLK, BLK], mybir.dt.float32, kind="ExternalOutput")
    out_cs = nc.dram_tensor(
        "out_cs", [1, cs_cols], mybir.dt.float32, kind="ExternalOutput"
    )

    # DRAM views per group: (128, m*CH), partition-major rows
    bases = np.cumsum([0] + ms).tolist()
    ev_views = [
        ev.ap()[P * bases[i] : P * bases[i + 1], :].rearrange(
            "(p m) d -> p (m d)", p=P
        )
        for i in range(ngrp)
    ]

    with ExitStack() as ctx:
        bufs = [
            ctx.enter_context(
                nc.sbuf_tensor(f"buf{i}", [P, mmax * CH], mybir.dt.bfloat16)
            )
            for i in range(NBUF)
        ]
        ones = ctx.enter_context(nc.sbuf_tensor("ones", [P, 1], mybir.dt.bfloat16))
        gsb = ctx.enter_context(nc.sbuf_tensor("gsb", [BLK, BLK], mybir.dt.float32))
        cssb = ctx.enter_context(
            nc.sbuf_tensor("cssb", [1, cs_cols], mybir.dt.float32)
        )
        gacc = ctx.enter_context(
            nc.psum_tensor("gacc", [BLK, BLK], mybir.dt.float32)
        )
        csacc = ctx.enter_context(
            nc.psum_tensor("csacc", [1, cs_cols], mybir.dt.float32)
        )
        dma_sems = [
            ctx.enter_context(nc.semaphore(f"dma_sem{i}")) for i in range(NBUF)
        ]
        ten_sem = ctx.enter_context(nc.semaphore("ten_sem"))
        ones_sem = ctx.enter_context(nc.semaphore("ones_sem"))
        copy_sem = ctx.enter_context(nc.semaphore("copy_sem"))
        odma_sem = ctx.enter_context(nc.semaphore("odma_sem"))
        odma2_sem = ctx.enter_context(nc.semaphore("odma2_sem"))
        copy2_sem = ctx.enter_context(nc.semaphore("copy2_sem"))
        block = ctx.enter_context(nc.Block())

        @block.gpsimd
        def _(g: bass.BassEngine):
            for i, m in enumerate(ms):
                if i == 1:
                    # after the first DMA is under way; needed only by the
                    # first colsum matmul, which runs much later
                    g.memset(ones.ap(), 1.0).then_inc(ones_sem, 1)
                if i >= NBUF:
                    # wait until tensor is done reading this buffer
                    g.wait_ge(ten_sem, i - NBUF + 1)
                buf = bufs[i % NBUF]
                # SWDGE DMA with fp32 -> bf16 cast in flight.  One
                # semaphore per buffer slot: a slot's sem has only one
                # DMA in flight at a time, so sem == 16*(k+1) proves
                # that DMA fully landed (per-SDMA-engine increments of
                # concurrent DMAs interleave on a shared sem).
                g.dma_start(out=buf.ap()[:, : m * CH], in_=ev_views[i]).then_inc(
                    dma_sems[i % NBUF], 16
                )

        @block.tensor
        def _(t: bass.BassEngine):
            total_g = sum(m // C for m in ms)
            total_cs = sum((m + CS_MAX - 1) // CS_MAX for m in ms)
            gi = ci = 0
            for i, m in enumerate(ms):
                t.wait_ge(dma_sems[i % NBUF], 16 * (i // NBUF + 1))
                buf = bufs[i % NBUF]
                last = None
                for j in range(m // C):
                    sl = buf.ap()[:, j * BLK : (j + 1) * BLK]
                    last = t.matmul(
                        gacc.ap(),
                        sl,
                        sl,
                        start=(gi == 0),
                        stop=(gi == total_g - 1),
                    )
                    gi += 1
                if i == 0:
                    t.wait_ge(ones_sem, 1)
                bview = buf.ap()[:, : m * CH].rearrange("p (m d) -> p m d", d=CH)
                for c0 in range(0, m, CS_MAX):
                    cn = min(CS_MAX, m - c0)
                    vs = bview[:, c0 : c0 + cn, D:CH]
                    last = t.matmul(
                        csacc.ap()[:, : cn * S],
                        ones.ap(),
                        vs,
                        start=(ci == 0),
                        stop=(ci == total_cs - 1),
                    )
                    ci += 1
                last.then_inc(ten_sem, 1)

        @block.vector
        def _(v: bass.BassEngine):
            v.wait_ge(ten_sem, ngrp)
            v.tensor_copy(gsb.ap(), gacc.ap()).then_inc(copy_sem, 1)

        @block.scalar
        def _(sc: bass.BassEngine):
            # out_cs path runs fully on ACT (copy + HWDGE DMA), parallel
            # with the DVE/SP out_g path
            sc.wait_ge(ten_sem, ngrp)
            sc.copy(cssb.ap(), csacc.ap()).then_inc(copy2_sem, 1)
            sc.wait_ge(copy2_sem, 1)  # DMA reads must see the copy's writes
            sc.dma_start(out=out_cs.ap(), in_=cssb.ap()).then_inc(odma2_sem, 16)
            sc.wait_ge(odma2_sem, 16)

        @block.sync
        def _(s: bass.BassEngine):
            s.wait_ge(copy_sem, 1)
            s.dma_start(out=out_g.ap(), in_=gsb.ap()).then_inc(odma_sem, 16)
            s.wait_ge(odma_sem, 16)

    return nc


def pack_inputs(embeddings, source_indicators, npad=NPAD):
    """(B,F,T,D)+(B,F,T,S) -> per-core padded interleaved (npad, 22)."""
    b = embeddings.shape[0]
    n = embeddings.shape[1] * embeddings.shape[2]
    e = np.asarray(embeddings, dtype=np.float32).reshape(b, n, D)
    v = np.asarray(source_indicators, dtype=np.float32).reshape(b, n, S)
    evp = np.zeros((b, npad, CH), dtype=np.float32)
    evp[:, :n, :D] = e
    evp[:, :n, D:] = v
    return evp


def reduce_outputs(res):
    """Per-core raw outputs -> (G_b, EtV_b, colsum_b) in float64."""
    out_g = np.asarray(res["out_g"], dtype=np.float64)
    out_cs = np.asarray(res["out_cs"], dtype=np.float64)
    g_b = np.zeros((D, D))
    etv_b = np.zeros((D, S))
    for c in range(C):
        blk = out_g[c * CH : (c + 1) * CH, c * CH : (c + 1) * CH]
        g_b += blk[:D, :D]
        etv_b += blk[:D, D:CH]
    colsum_b = out_cs.reshape(-1, S).sum(axis=0)
    return g_b, etv_b, colsum_b


_NC_CACHE = {}


def _get_nc():
    if "nc" not in _NC_CACHE:
        _NC_CACHE["nc"] = build_bass()
    return _NC_CACHE["nc"]


def kernel(embeddings, source_indicators):
    evp = pack_inputs(embeddings, source_indicators)
    nc = _get_nc()
    in_maps = [{"ev": np.ascontiguousarray(evp[b])} for b in range(B)]
    results = run_bass_kernel_spmd(nc, in_maps, list(range(B))).results

    loss = 0.0
    norms = []
    for b in range(B):
        g_b, etv_b, colsum_b = reduce_outputs(results[b])
        ety = etv_b / (colsum_b[None, :] + 1e-8)
        loss += float(np.sum(ety * ety))
        norms.append(float(np.sum(g_b * g_b)))
    norm_term = float(np.mean(norms))
    return np.float32(-loss / (norm_term + 1e-8))


# revision 17
# speedup vs baseline: 1.0909x; 1.0064x over previous
"""DeepClusteringLoss on 8 TRN2 NeuronCores.

loss = -sum_b ||E_b^T Y_b||_F^2 / (mean_b ||E_b^T E_b||_F^2 + 1e-8)
with Y = V / (colsum(V) + 1e-8), E: (B, N, D), V: (B, N, S), N = F*T.

Sharding: data-parallel over batch (8 batches -> 8 cores). Each core
reduces its 22.6 MB shard to a (110,110) Gram block matrix + (1,510)
column-sum vector on-device; the host sums diagonal blocks and combines
the per-batch scalars (a few hundred flops).

Device algorithm (per core), raw Bass (no Tile framework -> no multi-us
preamble/drain barriers):
  Host pre-interleaves E and V into EV = (N, 22) rows [e_0..e_19, v_0, v_1]
  and zero-pads N=257000 to 2010*128 = 257280 rows (zero rows are inert).
  The padded array is split into DMA groups of m_i row-chunks
  (ms = [510, 510, 510, 425, 55]; sum = 2010; the small last group keeps
  the tensor-engine tail exposed after the final DMA short).  Group i is
  viewed as (128, m_i*22): partition p holds m_i consecutive 22-float rows,
  so each DMA line is m_i*88 contiguous DRAM bytes.  One SWDGE DMA per
  group loads f32 -> bf16 (cast in flight; HBM still reads full fp32).
  Matmuls contract over the 128 partitions: a 110-column slice (5 chunks
  x 22) as both stationary and moving gives a (110,110) PSUM block whose
  five diagonal 22x22 blocks are sum_rows [e|v]^T [e|v] = [[E^T E, E^T V],
  [., V^T V]] for those rows; PSUM-accumulating all slices of all groups
  leaves full-batch Gram sums in the diagonal blocks.  colsum(V) comes
  from ones(128,1)^T @ V-columns (strided AP over <=255 chunks at a
  time), PSUM-accumulated into (1, 510).
"""

import sys

if "/opt/trn_rl_repo" not in sys.path:
    sys.path.insert(0, "/opt/trn_rl_repo")

from contextlib import ExitStack

import numpy as np

import concourse.bass as bass
from concourse import mybir
from concourse.bass_utils import run_bass_kernel_spmd

# Problem geometry (hardcoded; see spec)
B, F, T, D, S = 8, 257, 1000, 20, 2
N = F * T  # 257000
CH = D + S  # 22 interleaved columns per row
P = 128  # SBUF partitions
C = 5  # row-chunks fused per matmul (C*CH = 110 <= 128 stationary cols)
BLK = C * CH  # 110

# Row-chunks per DMA group (sum = 2010 -> NPAD = 257280, 0.1% padding).
# Medium groups keep the tensor engine close behind the DMA stream; the
# geometrically decaying tail lets the (DMA-gated, HAM-throttled) tensor
# engine drain its backlog before the stream ends, so almost no matmul
# work is exposed after the last DMA.
MS = [255] * 7 + [100, 60, 35, 20, 10]
NBUF = 3  # SBUF buffers for DMA/compute overlap
CS_MAX = 255  # chunks per colsum matmul (2*255 <= 512 fp32 PSUM bank)
NPAD = P * sum(MS)  # 257280


def build_bass(ms=None, n_cores=B):
    """Build the per-core raw-Bass SPMD program (same program on every
    core; only the input data differs)."""
    ms = list(MS if ms is None else ms)
    assert all(m % C == 0 for m in ms)
    npad = P * sum(ms)
    mmax = max(ms)
    cs_cols = min(mmax, CS_MAX) * S
    ngrp = len(ms)

    nc = bass.Bass("TRN2", debug=False, num_devices=n_cores)
    ev = nc.dram_tensor("ev", [npad, CH], mybir.dt.float32, kind="ExternalInput")
    out_g = nc.dram_tensor("out_g", [BLK, BLK], mybir.dt.float32, kind="ExternalOutput")
    out_cs = nc.dram_tensor(
        "out_cs", [1, cs_cols], mybir.dt.float32, kind="ExternalOutput"
    )

    # DRAM views per group: (128, m*CH), partition-major rows
    bases = np.cumsum([0] + ms).tolist()
    ev_views = [
        ev.ap()[P * bases[i] : P * bases[i + 1], :].rearrange(
            "(p m) d -> p (m d)", p=P
        )
        for i in range(ngrp)
    ]

    with ExitStack() as ctx:
        bufs = [
            ctx.enter_context(
                nc.sbuf_tensor(f"buf{i}", [P, mmax * CH], mybir.dt.bfloat16)
            )
            for i in range(NBUF)
        ]
        ones = ctx.enter_context(nc.sbuf_tensor("ones", [P, 1], mybir.dt.bfloat16))
        gsb = ctx.enter_context(nc.sbuf_tensor("gsb", [BLK, BLK], mybir.dt.float32))
        cssb = ctx.enter_context(
            nc.sbuf_tensor("cssb", [1, cs_cols], mybir.dt.float32)
        )
        gacc = ctx.enter_context(
            nc.psum_tensor("gacc", [BLK, BLK], mybir.dt.float32)
        )
        csacc = ctx.enter_context(
            nc.psum_tensor("csacc", [1, cs_cols], mybir.dt.float32)
        )
        dma_sems = [
            ctx.enter_context(nc.semaphore(f"dma_sem{i}")) for i in range(NBUF)
        ]
        ten_sem = ctx.enter_context(nc.semaphore("ten_sem"))
        ones_sem = ctx.enter_context(nc.semaphore("ones_sem"))
        copy_sem = ctx.enter_context(nc.semaphore("copy_sem"))
        odma_sem = ctx.enter_context(nc.semaphore("odma_sem"))
        odma2_sem = ctx.enter_context(nc.semaphore("odma2_sem"))
        copy2_sem = ctx.enter_context(nc.semaphore("copy2_sem"))
        block = ctx.enter_context(nc.Block())

        @block.gpsimd
        def _(g: bass.BassEngine):
            for i, m in enumerate(ms):
                if i == 1:
                    # after the first DMA is under way; needed only by the
                    # first colsum matmul, which runs much later
                    g.memset(ones.ap(), 1.0).then_inc(ones_sem, 1)
                if i >= NBUF:
                    # wait until tensor is done reading this buffer
                    g.wait_ge(ten_sem, i - NBUF + 1)
                buf = bufs[i % NBUF]
                # SWDGE DMA with fp32 -> bf16 cast in flight.  One
                # semaphore per buffer slot: a slot's sem has only one
                # DMA in flight at a time, so sem == 16*(k+1) proves
                # that DMA fully landed (per-SDMA-engine increments of
                # concurrent DMAs interleave on a shared sem).
                g.dma_start(out=buf.ap()[:, : m * CH], in_=ev_views[i]).then_inc(
                    dma_sems[i % NBUF], 16
                )

        @block.tensor
        def _(t: bass.BassEngine):
            total_g = sum(m // C for m in ms)
            total_cs = sum((m + CS_MAX - 1) // CS_MAX for m in ms)
            gi = ci = 0
            for i, m in enumerate(ms):
                t.wait_ge(dma_sems[i % NBUF], 16 * (i // NBUF + 1))
                buf = bufs[i % NBUF]
                last = None
                for j in range(m // C):
                    sl = buf.ap()[:, j * BLK : (j + 1) * BLK]
                    last = t.matmul(
                        gacc.ap(),
                        sl,
                        sl,
                        start=(gi == 0),
                        stop=(gi == total_g - 1),
                    )
                    gi += 1
                if i == 0:
                    t.wait_ge(ones_sem, 1)
                bview = buf.ap()[:, : m * CH].rearrange("p (m d) -> p m d", d=CH)
                for c0 in range(0, m, CS_MAX):
                    cn = min(CS_MAX, m - c0)
                    vs = bview[:, c0 : c0 + cn, D:CH]
                    last = t.matmul(
                        csacc.ap()[:, : cn * S],
                        ones.ap(),
                        vs,
                        start=(ci == 0),
                        stop=(ci == total_cs - 1),
                    )
                    ci += 1
                last.then_inc(ten_sem, 1)

        @block.vector
        def _(v: bass.BassEngine):
            # DVE does both PSUM -> SBUF copies (ACT would pay a ~1.3 us
            # activation-table load for its first ACTIVATE)
            v.wait_ge(ten_sem, ngrp)
            v.tensor_copy(gsb.ap(), gacc.ap()).then_inc(copy_sem, 1)
            v.tensor_copy(cssb.ap(), csacc.ap()).then_inc(copy2_sem, 1)

        @block.scalar
        def _(sc: bass.BassEngine):
            # ACT issues the out_cs HWDGE DMA, parallel with SP's out_g DMA
            sc.wait_ge(copy2_sem, 1)
            sc.dma_start(out=out_cs.ap(), in_=cssb.ap()).then_inc(odma2_sem, 16)
            sc.wait_ge(odma2_sem, 16)

        @block.sync
        def _(s: bass.BassEngine):
            s.wait_ge(copy_sem, 1)
            s.dma_start(out=out_g.ap(), in_=gsb.ap()).then_inc(odma_sem, 16)
            s.wait_ge(odma_sem, 16)

    return nc


def pack_inputs(embeddings, source_indicators, npad=NPAD):
    """(B,F,T,D)+(B,F,T,S) -> per-core padded interleaved (npad, 22)."""
    b = embeddings.shape[0]
    n = embeddings.shape[1] * embeddings.shape[2]
    e = np.asarray(embeddings, dtype=np.float32).reshape(b, n, D)
    v = np.asarray(source_indicators, dtype=np.float32).reshape(b, n, S)
    evp = np.zeros((b, npad, CH), dtype=np.float32)
    evp[:, :n, :D] = e
    evp[:, :n, D:] = v
    return evp


def reduce_outputs(res):
    """Per-core raw outputs -> (G_b, EtV_b, colsum_b) in float64."""
    out_g = np.asarray(res["out_g"], dtype=np.float64)
    out_cs = np.asarray(res["out_cs"], dtype=np.float64)
    g_b = np.zeros((D, D))
    etv_b = np.zeros((D, S))
    for c in range(C):
        blk = out_g[c * CH : (c + 1) * CH, c * CH : (c + 1) * CH]
        g_b += blk[:D, :D]
        etv_b += blk[:D, D:CH]
    colsum_b = out_cs.reshape(-1, S).sum(axis=0)
    return g_b, etv_b, colsum_b


_NC_CACHE = {}


def _get_nc():
    if "nc" not in _NC_CACHE:
        _NC_CACHE["nc"] = build_bass()
    return _NC_CACHE["nc"]


def kernel(embeddings, source_indicators):
    evp = pack_inputs(embeddings, source_indicators)
    nc = _get_nc()
    in_maps = [{"ev": np.ascontiguousarray(evp[b])} for b in range(B)]
    results = run_bass_kernel_spmd(nc, in_maps, list(range(B))).results

    loss = 0.0
    norms = []
    for b in range(B):
        g_b, etv_b, colsum_b = reduce_outputs(results[b])
        ety = etv_b / (colsum_b[None, :] + 1e-8)
        loss += float(np.sum(ety * ety))
        norms.append(float(np.sum(g_b * g_b)))
    norm_term = float(np.mean(norms))
    return np.float32(-loss / (norm_term + 1e-8))
